# revision 29
# baseline (speedup 1.0000x reference)
"""Trainium2 Bass kernel: batched cosine-similarity relation matrix.

Computes out[b,i,j,m,n] = <q_hat[b,i,m,:], s_hat[b,j,n,:]> where q_hat/s_hat
are L2-normalized along k (torch F.normalize semantics, eps=1e-12).

Shapes (hardcoded): query/support [4, 25, 128, 64] f32 -> out [4, 25, 25, 128, 128] f32.

Sharding: flat and exactly balanced. The global output is [(b i m), (j n)] =
[12800, 3200]; core c computes rows [1600c, 1600c+1600) (all within b = c//2,
since each b spans 3200 rows). Each core's slice is [1600, 3200] — no padding,
no imbalance, no communication.

Device pipeline per core (inputs arrive pre-transposed, contraction dim k on
partitions -- a pure host-side layout change):
  1. Load qT [64, 1600] / sT [64, 3200] fp32 (head chunks via HWDGE for fast
     start, tails via SWDGE to stay off the output ring).
  2. Normalize in column chunks: square (Pool), ones-matmul (PE) -> sum_k sq
     replicated across psum partitions, sqrt(+1e-24) (ACT), reciprocal (DVE),
     multiply-and-cast to fp16 (DVE).
  3. 13 row-groups x 7 matmuls: psum[128, 512] = qT16[64,128].T @ sT16[64,512]
     (fp16 in, fp32 accumulate).
  4. PSUM->SBUF copies (split ACT/DVE) cast to fp16; output DMA per row-group
     is a dense [128, 3200] fp16 block = 6.4KB contiguous per partition --
     maximal descriptor efficiency. Host transposes (i m j n) -> (i j m n) and
     upcasts to f32 (layout glue only, like the input pre-transpose).
"""

import os

import numpy as np

import concourse.bacc as bacc
import concourse.bass as bass
import concourse.mybir as mybir
import concourse.tile as tile
from concourse.bass_utils import run_bass_kernel_spmd

B, I, M, K = 4, 25, 128, 64
J, N = 25, 128
RPC = 1600  # q-rows per core = B*I*M / 8
CW = J * N  # 3200 output columns
NCORES = 8
# 13 row-groups: 12 x 128 + 1 x 64 = 1600
GROUPS = [(g * 128, 128) for g in range(12)] + [(1536, 64)]

# Stash of the most recent BassKernelResults (test.py reads exec_time_ns).
last_results = None

_nc_cache = {}

# Version tag for the production build: a dummy input of this width makes the
# jitted HLO (and thus the terminal-side neuron compile-cache key, which
# ignores the embedded BIR) unique per kernel revision. Bump on every edit.
KVER = 75


def _build_nc(
    mm_dtype=mybir.dt.float16,
    ob_bufs=4,
    mm_bufs=5,
    copy_pattern="av",
    reps=1,
    bench_tag=0,
    out_eng_pattern="s",
    ndma=3,
    sq_eng="a",
    mul_eng="v",
    tail_eng=None,
    prep_fast=2,
    ramp_groups=0,
    inp_bufs=2,
):
    f32 = mybir.dt.float32
    f16 = mybir.dt.float16
    nc = bacc.Bacc(trn_type="TRN2")
    qT_d = nc.dram_tensor("qT", [K, RPC], f16, kind="ExternalInput")
    sT_d = nc.dram_tensor("sT", [K, CW], f16, kind="ExternalInput")
    out = nc.dram_tensor("out", [RPC, CW], f16, kind="ExternalOutput")
    if bench_tag:
        # Extra dummy input of a distinctive size so the jitted HLO (and thus
        # the neuron compile-cache key) differs per variant -- the cache key
        # ignores the embedded BIR.
        pad_d = nc.dram_tensor("pad", [1, bench_tag], f32, kind="ExternalInput")

    # Column blocks per row-group. Ramp groups use narrow-first blocks with
    # per-block DMAs so the output stream starts early; steady groups use
    # 512-wide blocks and one dense DMA per group.
    blocks_ramp = [(0, 128), (128, 128), (256, 256), (512, 512), (1024, 512),
                   (1536, 512), (2048, 512), (2560, 512), (3072, 128)]
    blocks_steady = [(0, 512), (512, 512), (1024, 512), (1536, 512),
                     (2048, 512), (2560, 512), (3072, 128)]

    out_engs = {"s": nc.sync, "a": nc.scalar, "v": nc.vector, "p": nc.gpsimd}

    with tile.TileContext(nc) as tc:
        with (
            tc.tile_pool(name="const", bufs=1) as const,
            tc.tile_pool(name="inp", bufs=inp_bufs) as inp,
            tc.tile_pool(name="mmp", bufs=mm_bufs, space="PSUM") as mmp,
            tc.tile_pool(name="npp", bufs=2, space="PSUM") as npp,
            tc.tile_pool(name="obp", bufs=ob_bufs) as obp,
        ):
            eps_t = const.tile([128, 1], f32)
            nc.vector.memset(eps_t, 1e-24)
            ones_t = const.tile([K, K], f32)
            nc.vector.memset(ones_t, 1.0)
            # Dummy Sqrt up front: absorbs the ACT table switch to
            # "sqrt_and_others" (which also contains copy and square, so no
            # further table loads) on an instruction with few waits.
            warm = const.tile([128, 1], f32)
            nc.scalar.activation(
                out=warm,
                in_=eps_t,
                func=mybir.ActivationFunctionType.Sqrt,
                bias=eps_t,
            )

            if bench_tag:
                pad_sb = const.tile([1, bench_tag], f32)
                nc.gpsimd.dma_start(out=pad_sb, in_=pad_d[:])

            # Inputs arrive pre-cast to fp16 (host-side): qT16 is used as
            # the matmul lhsT directly; sT_raw is normalized into sT16.
            qT16 = inp.tile([K, RPC], mm_dtype)
            sT_raw = inp.tile([K, CW], mm_dtype)
            sT16 = inp.tile([K, CW], mm_dtype)

            def _body():
                # s loads go via the sync HWDGE ring (idle before the output
                # stream starts) in pieces so each prep chunk starts as soon
                # as its slice lands; the q tail goes via SWDGE (Pool) to
                # keep the rings parallel.
                nc.sync.dma_start(out=sT_raw[:, 0:512], in_=sT_d[:, 0:512])
                nc.sync.dma_start(out=qT16[:, 0:128], in_=qT_d[:, 0:128])
                nc.sync.dma_start(out=sT_raw[:, 512:1792], in_=sT_d[:, 512:1792])
                nc.sync.dma_start(out=sT_raw[:, 1792:CW], in_=sT_d[:, 1792:CW])
                nc.gpsimd.dma_start(out=qT16[:, 128:RPC], in_=qT_d[:, 128:RPC])

                s_chunks = blocks_ramp if ramp_groups else blocks_steady

                # q is NOT pre-normalized: qT16 is a plain fp16 cast, and the
                # per-row 1/|q| is folded into the PSUM->SBUF copies as a
                # per-partition scale. Row-norms for group g come from a tiny
                # ones-column matmul: qsq[:, rows].T @ ones[64,1] -> [rows,1].
                qsq = inp.tile([K, RPC], f32)
                np_q = npp.tile([M, 16], f32, tag="npq", name="np_q", bufs=1)
                qinv = inp.tile([M, 16], f32)

                def q_cast_sq(c0, w, eng):
                    e = nc.vector if eng == "v" else nc.gpsimd
                    e.tensor_mul(
                        qsq[:, c0 : c0 + w],
                        qT16[:, c0 : c0 + w],
                        qT16[:, c0 : c0 + w],
                    )

                def q_rowmms(g_lo, g_hi):
                    for g in range(g_lo, g_hi):
                        r0, rows = GROUPS[g]
                        nc.tensor.matmul(
                            np_q[:rows, g : g + 1],
                            lhsT=qsq[:, r0 : r0 + rows],
                            rhs=ones_t[:, 0:1],
                            start=True,
                            stop=True,
                        )

                def q_inv(g_lo, g_hi, rows=M):
                    nc.scalar.activation(
                        out=qinv[:rows, g_lo:g_hi],
                        in_=np_q[:rows, g_lo:g_hi],
                        func=mybir.ActivationFunctionType.Sqrt,
                        bias=eps_t[:rows],
                    )
                    nc.vector.reciprocal_approx_fast(
                        out=qinv[:rows, g_lo:g_hi], in_=qinv[:rows, g_lo:g_hi]
                    )

                def prep_s(c):
                    """Normalize s cols [c0, c0+w) along k, cast to fp16.

                    Stage engines chosen so consecutive chunks pipeline:
                    ACT (square, sqrt) / PE (ones-MM) / DVE (recip, mul)."""
                    c0, w = s_chunks[c]
                    xs = sT_raw[:, c0 : c0 + w]
                    sq_c = inp.tile([K, 512], f32, tag="sq", name="sq_c", bufs=3)
                    if sq_eng == "a":
                        nc.scalar.activation(
                            out=sq_c[:, :w],
                            in_=xs,
                            func=mybir.ActivationFunctionType.Square,
                        )
                    else:
                        (nc.vector if sq_eng == "v" else nc.gpsimd).tensor_mul(
                            sq_c[:, :w], xs, xs
                        )
                    # ones[64,64].T @ sq[64, w] -> psum[64, w]: every partition
                    # row holds sum_k sq[k, c] = ||s_c||^2.
                    np_t = npp.tile([K, 512], f32, tag="np", name="np_t")
                    nc.tensor.matmul(
                        np_t[:, :w],
                        lhsT=ones_t,
                        rhs=sq_c[:, :w],
                        start=True,
                        stop=True,
                    )
                    inv_c = inp.tile([K, 512], f32, tag="inv", name="inv_c", bufs=3)
                    # sqrt(sumsq + 1e-24): zero cols -> norm 1e-12, matching
                    # the reference's max(norm, 1e-12), no inf/nan.
                    nc.scalar.activation(
                        out=inv_c[:, :w],
                        in_=np_t[:, :w],
                        func=mybir.ActivationFunctionType.Sqrt,
                        bias=eps_t[:K],
                    )
                    # 1/norm at ~18 correct bits (feeds fp16; plenty), ~5x
                    # cheaper than exact reciprocal. Inputs are >= 1e-12 so
                    # the undefined edge cases can't occur.
                    nc.vector.reciprocal_approx_fast(
                        out=inv_c[:, :w], in_=inv_c[:, :w]
                    )
                    (nc.vector if mul_eng == "v" else nc.gpsimd).tensor_mul(
                        sT16[:, c0 : c0 + w], xs, inv_c[:, :w]
                    )

                # Fast q chain for group 0 (DVE), then kick off the first s
                # chunks; the q tail casts/squares run on Pool in background.
                q_cast_sq(0, 128, "v")
                q_rowmms(0, 1)
                q_inv(0, 1)
                for c in range(min(prep_fast, len(s_chunks))):
                    prep_s(c)
                q_cast_sq(128, 768, "p")
                q_cast_sq(896, 704, "p")

                it = 0
                od = 0
                # Steady groups: fire the output DMA for a column span as
                # soon as its last block is copied (ndma spans per group).
                if ndma == 1:
                    dma_after = {len(blocks_steady) - 1: (0, CW)}
                elif ndma == 2:
                    dma_after = {2: (0, 1536), len(blocks_steady) - 1: (1536, CW - 1536)}
                else:
                    dma_after = {1: (0, 1024), 3: (1024, 1024),
                                 len(blocks_steady) - 1: (2048, CW - 2048)}
                for gi, (r0, rows) in enumerate(GROUPS):
                    ramp = gi < ramp_groups
                    if not ramp:
                        big = obp.tile([M, CW], f16, tag="ob", name="big")
                    qsc = qinv[:rows, gi : gi + 1]
                    for bi, (c0, w) in enumerate(blocks_ramp if ramp else blocks_steady):
                        if gi == 0:
                            # Software-pipelined s prep: stay `prep_fast`
                            # chunks ahead of the consuming block.
                            nxt = bi + prep_fast
                            if prep_fast <= nxt < len(s_chunks):
                                prep_s(nxt)
                        ps = mmp.tile([M, 512], f32, tag="mm", name="ps")
                        nc.tensor.matmul(
                            ps[:rows, :w],
                            lhsT=qT16[:, r0 : r0 + rows],
                            rhs=sT16[:, c0 : c0 + w],
                            start=True,
                            stop=True,
                        )
                        if ramp:
                            o_tile = obp.tile(
                                [M, 512], f16, tag="obs", name="o_tile", bufs=8
                            )
                            o_t = o_tile[:rows, :w]
                        else:
                            o_t = big[:rows, c0 : c0 + w]
                        # PSUM->SBUF cast-copies split between ACT and DVE
                        # (optionally Pool for narrow tails), applying the
                        # per-partition 1/|q| scale in the same pass.
                        eng = copy_pattern[it % len(copy_pattern)]
                        if tail_eng and w <= 128 and not ramp:
                            eng = tail_eng
                        if eng == "a":
                            nc.scalar.activation(
                                out=o_t,
                                in_=ps[:rows, :w],
                                func=mybir.ActivationFunctionType.Copy,
                                scale=qsc,
                            )
                        elif eng == "p":
                            nc.gpsimd.tensor_scalar_mul(o_t, ps[:rows, :w], qsc)
                        else:
                            nc.vector.tensor_scalar_mul(o_t, ps[:rows, :w], qsc)
                        if ramp:
                            # Ramp: small per-block DMAs so the output stream
                            # starts as soon as the first block is ready.
                            oeng = out_engs[out_eng_pattern[od % len(out_eng_pattern)]]
                            oeng.dma_start(
                                out=out[r0 : r0 + rows, c0 : c0 + w], in_=o_t
                            )
                            od += 1
                        elif bi in dma_after:
                            d0, dw = dma_after[bi]
                            oeng = out_engs[out_eng_pattern[od % len(out_eng_pattern)]]
                            oeng.dma_start(
                                out=out[r0 : r0 + rows, d0 : d0 + dw],
                                in_=big[:rows, d0 : d0 + dw],
                            )
                            od += 1
                        it += 1
                    if gi == 0:
                        # Row-norm matmuls for the remaining groups, emitted
                        # at points where their qsq dependency is ready so
                        # they don't stall PE between group matmuls.
                        q_rowmms(1, 7)
                        q_inv(1, 7)
                    elif gi == 2:
                        q_rowmms(7, 13)
                        q_inv(7, 12)
                        q_inv(12, 13, rows=64)

            if reps > 1:
                # Benchmark mode: repeat the whole pipeline in a HW loop so
                # per-call tunnel overhead can be cancelled by slope fitting.
                with tc.For_i(0, reps, 1):
                    _body()
            else:
                _body()
    nc.compile()
    return nc


def _get_nc():
    if "nc" not in _nc_cache:
        _nc_cache["nc"] = _build_nc(bench_tag=KVER)
    return _nc_cache["nc"]


def _shard_inputs(query, support):
    # fp16 on the wire: halves input DMA and lets the device skip the cast;
    # norms are computed from the same rounded values the matmul consumes.
    q = np.asarray(query, dtype=np.float32).astype(np.float16)
    s = np.asarray(support, dtype=np.float32).astype(np.float16)
    qflat = q.reshape(B, I * M, K)
    in_maps = []
    for c in range(NCORES):
        b, h = divmod(c, 2)
        # [rows, K] -> [K, rows]: contraction dim on partitions, so the
        # device needs no transposes (pure host-side relayout).
        qc = np.ascontiguousarray(qflat[b, h * RPC : (h + 1) * RPC].T)
        sc = np.ascontiguousarray(s[b].transpose(2, 0, 1).reshape(K, CW))
        in_maps.append({"qT": qc, "sT": sc})
    return in_maps


def kernel(query, support):
    global last_results
    nc = _get_nc()
    in_maps = [
        dict(m, pad=np.zeros((1, KVER), np.float32))
        for m in _shard_inputs(query, support)
    ]
    trace = bool(int(os.environ.get("BASS_KERNEL_TRACE", "0")))
    if not trace:
        # The axon client here has no NTFF hook; an external BASS_TRACE=1
        # would crash run_bass_kernel_spmd on a missing import.
        os.environ.setdefault("BASS_NEVER_TRACE", "1")
    res = run_bass_kernel_spmd(
        nc,
        in_maps,
        core_ids=list(range(NCORES)),
        trace=trace,
    )
    last_results = res
    full = np.empty((B, I, J, M, N), dtype=np.float32)
    for b in range(B):
        # [3200, 3200] fp16 block for this b: rows (i m), cols (j n).
        blk = np.concatenate(
            [res.results[2 * b]["out"], res.results[2 * b + 1]["out"]], axis=0
        )
        full[b] = (
            blk.reshape(I, M, J, N).transpose(0, 2, 1, 3).astype(np.float32)
        )
    return full
